# revision 1
# baseline (speedup 1.0000x reference)
"""Trainium2 Bass kernel for a 2-layer cosine-similarity attention GCN.

Reference math (per (b,h) slice, two chained blocks):
    xn = x / max(||x||_row, eps)
    A  = softmax((xn @ xn^T) / max(alpha, 0.01), axis=-1)
    out = relu((A @ x) @ W^T + x)

Shapes: x [4, 4, 4096, 64] fp32; W [64, 64]. B*H = 16 slices sharded as
2 slices per NeuronCore across 8 cores (fully independent, no collectives).

Kernel strategy (per core, 2 pairs x 2 blocks, all on-chip):
  - logits are cosine sims in [-1,1]*scale -> softmax without max-subtraction:
    P = exp(S*scale) / Z with Z = rowsum. Z is obtained for free by
    augmenting V = x with a ones column and computing U = E @ [x, 1].
  - E tiles are produced directly in [j, i] orientation (j on partitions) so
    the E @ x matmul contracts over partitions; the full softmax matrix is
    never materialized in HBM.
  - division by Z is deferred past the (U @ W^T) matmul (a per-row scale
    commutes with right-multiplication), applied after a PE transpose where
    Z sits on the partition axis.
  - row 1/||x|| uses a fast inverse sqrt (bit trick + 3 Newton steps) on the
    vector engine so the scalar engine only ever runs Exp (no activation
    table switches).
"""

import numpy as np

import concourse.bacc as bacc
import concourse.tile as tile
from concourse import mybir
from concourse.bass_utils import run_bass_kernel_spmd
from concourse.masks import make_identity

F32 = mybir.dt.float32
U32 = mybir.dt.uint32
I32 = mybir.dt.int32
BF16 = mybir.dt.bfloat16
AF = mybir.ActivationFunctionType
ALU = mybir.AluOpType

P = 128
D = 64
N_CORES = 8


def build_nc(scales, n_rows=4096, npairs=2):
    nblocks = len(scales)
    NT = n_rows // P              # row tiles per pair
    CHW = min(1024, n_rows)       # i-chunk width (ACT call width)
    NCH = n_rows // CHW           # chunks
    HALF = 512                    # fp32 PSUM bank width (matmul free dim)
    NH = CHW // HALF

    nc = bacc.Bacc("TRN2", target_bir_lowering=False, debug=False, num_devices=N_CORES)
    xin = nc.dram_tensor("xin", [npairs, n_rows, D], F32, kind="ExternalInput").ap()
    wts = [
        nc.dram_tensor(f"w{i}t", [D, D], F32, kind="ExternalInput").ap()
        for i in range(nblocks)
    ]
    out = nc.dram_tensor("out", [npairs, n_rows, D], F32, kind="ExternalOutput").ap()

    xin_t = xin.rearrange("p (t pp) d -> p pp t d", pp=P)  # [np, 128, NT, 64]
    out_t = out.rearrange("p (t pp) d -> p pp t d", pp=P)

    with tile.TileContext(nc) as tc:
        with (
            tc.tile_pool(name="singles", bufs=1) as singles,
            tc.tile_pool(name="stats", bufs=2) as stats,
            tc.tile_pool(name="tmp", bufs=3) as tmp,
            tc.tile_pool(name="epool", bufs=6) as epool,
            tc.tile_pool(name="fin", bufs=2) as fin,
            tc.tile_pool(name="ps_big", bufs=2, space="PSUM") as ps_big,
            tc.tile_pool(name="ps_u", bufs=2, space="PSUM") as ps_u,
        ):
            ident16 = singles.tile([P, P], BF16, tag="ident16")
            make_identity(nc, ident16[:])
            identf = singles.tile([P, P], F32, tag="identf")
            make_identity(nc, identf[:])

            wf32 = []
            for i in range(nblocks):
                wtmp = singles.tile([D, D], F32, tag=f"wtmp{i}", name=f"wtmp{i}")
                nc.sync.dma_start(wtmp[:], wts[i])
                wf32.append(wtmp)

            # Persistent per (pair, block) state. xnt_all packs pair p's
            # normalized-transposed rows at partitions [64p, 64p+64) so the
            # two pairs' S-matmuls occupy different PE row groups and run
            # concurrently.
            xb = {}    # block input, row-major fp32 [128, NT, 64]
            xb16 = {}  # bf16 copy + ones column (U-matmul stationary operand)
            xnt_all = {}
            for blk in range(nblocks):
                xnt_all[blk] = singles.tile(
                    [P, n_rows], BF16, tag=f"xnt_{blk}", name=f"xnt_{blk}"
                )
            for p in range(npairs):
                for blk in range(nblocks):
                    xb[p, blk] = singles.tile([P, NT, D], F32, tag=f"xb_{p}_{blk}", name=f"xb_{p}_{blk}")
                    xb16[p, blk] = singles.tile(
                        [P, NT, D + 1], BF16, tag=f"xb16_{p}_{blk}", name=f"xb16_{p}_{blk}"
                    )
                    nc.vector.memset(xb16[p, blk][:, :, D : D + 1], 1.0)

            for p in range(npairs):
                nc.sync.dma_start(xb[p, 0][:], xin_t[p])

            MAGIC = 0x5F3759DF

            def prep(p, blk):
                """Row norms -> 1/||x||, normalized bf16 rows, PE-transpose
                into xnt. Also casts xb -> xb16 for block 0 (later blocks get
                the cast fused into the previous block's epilogue)."""
                s_all = stats.tile([P, NT], F32, tag="s_all")
                for b in range(NT):
                    xsl = xb[p, blk][:, b, :]
                    if blk == 0:
                        nc.vector.tensor_copy(xb16[p, blk][:, b, 0:D], xsl)
                    sq = tmp.tile([P, D], F32, tag="sq")
                    nc.vector.tensor_mul(sq[:], xsl, xsl)
                    nc.vector.reduce_sum(
                        s_all[:, b : b + 1], sq[:], axis=mybir.AxisListType.X
                    )
                nc.vector.tensor_scalar_max(s_all[:], s_all[:], 1e-24)
                # rinv = s^-0.5 via fast-inverse-sqrt seed + 3 Newton steps.
                # seed_bits = MAGIC - (bits(s) >> 1), computed overflow-free:
                # t = bits >> 1; v = t - MAGIC (negative int32); seed = ~v + 1
                r = stats.tile([P, NT], F32, tag="rinv")
                s_i = s_all[:].bitcast(I32)
                r_i = r[:].bitcast(I32)
                nc.vector.tensor_scalar(
                    out=r_i, in0=s_i, scalar1=1, scalar2=None,
                    op0=ALU.logical_shift_right,
                )
                nc.vector.tensor_scalar(
                    out=r_i, in0=r_i, scalar1=MAGIC, scalar2=None,
                    op0=ALU.subtract,
                )
                nc.vector.tensor_scalar(
                    out=r_i, in0=r_i, scalar1=-1, scalar2=None,
                    op0=ALU.bitwise_xor,
                )
                nc.vector.tensor_scalar(
                    out=r_i, in0=r_i, scalar1=1, scalar2=None,
                    op0=ALU.add,
                )
                t1 = stats.tile([P, NT], F32, tag="nt1")
                for _ in range(3):
                    nc.vector.tensor_mul(t1[:], r[:], r[:])
                    nc.vector.tensor_mul(t1[:], t1[:], s_all[:])
                    nc.vector.tensor_scalar(
                        out=t1[:],
                        in0=t1[:],
                        scalar1=-0.5,
                        scalar2=1.5,
                        op0=ALU.mult,
                        op1=ALU.add,
                    )
                    nc.vector.tensor_mul(r[:], r[:], t1[:])
                lo = D * p
                for b in range(NT):
                    xn16 = tmp.tile([P, D], BF16, tag="xn16")
                    nc.vector.tensor_scalar_mul(
                        xn16[:], xb[p, blk][:, b, :], r[:, b : b + 1]
                    )
                    pst = ps_big.tile([P, P], BF16, tag="big")
                    nc.tensor.transpose(pst[lo : lo + D, :], xn16[:], ident16[:])
                    nc.vector.tensor_copy(
                        xnt_all[blk][lo : lo + D, b * P : (b + 1) * P],
                        pst[lo : lo + D, :],
                    )

            def process_both(blk, scale, last):
                for a in range(NCH):
                    U = {}
                    for p in range(npairs):
                        U[p] = ps_u.tile([D + 1, CHW], F32, tag="U", name=f"U_{blk}_{a}_{p}")
                    def u_mms(p, b, E_b):
                        for h in range(NH):
                            nc.tensor.matmul(
                                U[p][:, h * HALF : (h + 1) * HALF],
                                lhsT=xb16[p, blk][:, b, :],
                                rhs=E_b[:, h * HALF : (h + 1) * HALF],
                                start=(b == 0),
                                stop=(b == NT - 1),
                            )

                    # U-matmuls run one b-iteration behind the S/exp pipeline
                    # so their E input is already materialized when they reach
                    # the PE queue head (no fine-grained PE stalls -> HAM can
                    # reach full clock).
                    E_prev = None
                    for b in range(NT):
                        E = {}
                        for p in range(npairs):
                            lo = D * p
                            xnt_p = xnt_all[blk][lo : lo + D, :]
                            S = ps_big.tile([P, CHW], F32, tag="big")
                            for h in range(NH):
                                nc.tensor.matmul(
                                    S[:, h * HALF : (h + 1) * HALF],
                                    lhsT=xnt_p[:, b * P : (b + 1) * P],
                                    rhs=xnt_p[
                                        :, a * CHW + h * HALF : a * CHW + (h + 1) * HALF
                                    ],
                                    start=True,
                                    stop=True,
                                )
                            E[p] = epool.tile([P, CHW], BF16, tag="E", name=f"E_{b}_{p}")
                            nc.scalar.activation(E[p][:], S[:], AF.Exp, scale=scale)
                        if E_prev is not None:
                            for p in range(npairs):
                                u_mms(p, b - 1, E_prev[p])
                        E_prev = E
                    for p in range(npairs):
                        u_mms(p, NT - 1, E_prev[p])
                    # chunk epilogue per pair: G = W @ U[:64], pack [G; Z],
                    # transpose, then out = relu(G/Z + x)
                    for p in range(npairs):
                        UTf = fin.tile([D, CHW], F32, tag="UTf")
                        nc.vector.tensor_copy(UTf[:], U[p][0:D, :])
                        G = ps_big.tile([D, CHW], F32, tag="big")
                        for h in range(NH):
                            nc.tensor.matmul(
                                G[:, h * HALF : (h + 1) * HALF],
                                lhsT=wf32[blk][:],
                                rhs=UTf[:, h * HALF : (h + 1) * HALF],
                                start=True,
                                stop=True,
                            )
                        GZ = fin.tile([D + 1, CHW], F32, tag="GZ")
                        nc.vector.tensor_copy(GZ[0:D, :], G[:])
                        nc.vector.tensor_copy(GZ[D : D + 1, :], U[p][D : D + 1, :])
                        for t in range(CHW // P):
                            gi = a * (CHW // P) + t
                            T = ps_big.tile([P, D + 1], F32, tag="big")
                            nc.tensor.transpose(
                                T[:],
                                GZ[:, t * P : (t + 1) * P],
                                identf[0 : D + 1, 0 : D + 1],
                            )
                            rz = tmp.tile([P, 1], F32, tag="rz")
                            nc.vector.reciprocal(rz[:], T[:, D : D + 1])
                            tmpo = tmp.tile([P, D], F32, tag="tmpo")
                            nc.vector.tensor_scalar_mul(tmpo[:], T[:, 0:D], rz[:])
                            nc.vector.tensor_add(tmpo[:], tmpo[:], xb[p, blk][:, gi, :])
                            if not last:
                                dst = xb[p, blk + 1][:, gi, :]
                                nc.vector.tensor_scalar_max(dst, tmpo[:], 0.0)
                                nc.vector.tensor_copy(
                                    xb16[p, blk + 1][:, gi, 0:D], dst
                                )
                            else:
                                oo = tmp.tile([P, D], F32, tag="oo")
                                nc.vector.tensor_scalar_max(oo[:], tmpo[:], 0.0)
                                nc.sync.dma_start(out_t[p][:, gi, :], oo[:])

            for blk in range(nblocks):
                xnt_all[blk] = singles.tile(
                    [P, n_rows], BF16, tag=f"xnt_{blk}", name=f"xnt_{blk}"
                )
            for p in range(npairs):
                for blk in range(nblocks):
                    xb[p, blk] = singles.tile([P, NT, D], F32, tag=f"xb_{p}_{blk}", name=f"xb_{p}_{blk}")
                    xb16[p, blk] = singles.tile(
                        [P, NT, D + 1], BF16, tag=f"xb16_{p}_{blk}", name=f"xb16_{p}_{blk}"
                    )
                    nc.vector.memset(xb16[p, blk][:, :, D : D + 1], 1.0)

            for p in range(npairs):
                nc.sync.dma_start(xb[p, 0][:], xin_t[p])

            MAGIC = 0x5F3759DF

            def prep(p, blk):
                """Row norms -> 1/||x||, normalized bf16 rows, PE-transpose
                into xnt. Also casts xb -> xb16 for block 0 (later blocks get
                the cast fused into the previous block's epilogue)."""
                s_all = stats.tile([P, NT], F32, tag="s_all")
                for b in range(NT):
                    xsl = xb[p, blk][:, b, :]
                    if blk == 0:
                        nc.vector.tensor_copy(xb16[p, blk][:, b, 0:D], xsl)
                    sq = tmp.tile([P, D], F32, tag="sq")
                    nc.vector.tensor_mul(sq[:], xsl, xsl)
                    nc.vector.reduce_sum(
                        s_all[:, b : b + 1], sq[:], axis=mybir.AxisListType.X
                    )
                nc.vector.tensor_scalar_max(s_all[:], s_all[:], 1e-24)
                # rinv = s^-0.5 via fast-inverse-sqrt seed + 3 Newton steps.
                # seed_bits = MAGIC - (bits(s) >> 1), computed overflow-free:
                # t = bits >> 1; v = t - MAGIC (negative int32); seed = ~v + 1
                r = stats.tile([P, NT], F32, tag="rinv")
                s_i = s_all[:].bitcast(I32)
                r_i = r[:].bitcast(I32)
                nc.vector.tensor_scalar(
                    out=r_i, in0=s_i, scalar1=1, scalar2=None,
                    op0=ALU.logical_shift_right,
                )
                nc.vector.tensor_scalar(
                    out=r_i, in0=r_i, scalar1=MAGIC, scalar2=None,
                    op0=ALU.subtract,
                )
                nc.vector.tensor_scalar(
                    out=r_i, in0=r_i, scalar1=-1, scalar2=None,
                    op0=ALU.bitwise_xor,
                )
                nc.vector.tensor_scalar(
                    out=r_i, in0=r_i, scalar1=1, scalar2=None,
                    op0=ALU.add,
                )
                t1 = stats.tile([P, NT], F32, tag="nt1")
                for _ in range(3):
                    nc.vector.tensor_mul(t1[:], r[:], r[:])
                    nc.vector.tensor_mul(t1[:], t1[:], s_all[:])
                    nc.vector.tensor_scalar(
                        out=t1[:],
                        in0=t1[:],
                        scalar1=-0.5,
                        scalar2=1.5,
                        op0=ALU.mult,
                        op1=ALU.add,
                    )
                    nc.vector.tensor_mul(r[:], r[:], t1[:])
                lo = D * p
                for b in range(NT):
                    xn16 = tmp.tile([P, D], BF16, tag="xn16")
                    nc.vector.tensor_scalar_mul(
                        xn16[:], xb[p, blk][:, b, :], r[:, b : b + 1]
                    )
                    pst = ps_big.tile([P, P], BF16, tag="big")
                    nc.tensor.transpose(pst[lo : lo + D, :], xn16[:], ident16[:])
                    nc.vector.tensor_copy(
                        xnt_all[blk][lo : lo + D, b * P : (b + 1) * P],
                        pst[lo : lo + D, :],
                    )

            def process(p, blk, scale, last):
                xnt_c = xnt[p, blk]
                xb16_c = xb16[p, blk]
                for a in range(NCH):
                    U = ps_u.tile([D + 1, CHW], F32, tag="U")
                    for b in range(NT):
                        S = ps_big.tile([P, CHW], F32, tag="big")
                        for h in range(NH):
                            nc.tensor.matmul(
                                S[:, h * HALF : (h + 1) * HALF],
                                lhsT=xnt_c[:, b * P : (b + 1) * P],
                                rhs=xnt_c[
                                    :, a * CHW + h * HALF : a * CHW + (h + 1) * HALF
                                ],
                                start=True,
                                stop=True,
                            )
                        E = epool.tile([P, CHW], BF16, tag="E")
                        nc.scalar.activation(E[:], S[:], AF.Exp, scale=scale)
                        for h in range(NH):
                            nc.tensor.matmul(
                                U[:, h * HALF : (h + 1) * HALF],
                                lhsT=xb16_c[:, b, :],
                                rhs=E[:, h * HALF : (h + 1) * HALF],
                                start=(b == 0),
                                stop=(b == NT - 1),
                            )
                    # chunk epilogue: G = W @ U[:64], pack [G; Z], transpose,
                    # then out = relu(G/Z + x)
                    UTf = fin.tile([D, CHW], F32, tag="UTf")
                    nc.vector.tensor_copy(UTf[:], U[0:D, :])
                    G = ps_big.tile([D, CHW], F32, tag="big")
                    for h in range(NH):
                        nc.tensor.matmul(
                            G[:, h * HALF : (h + 1) * HALF],
                            lhsT=wf32[blk][:],
                            rhs=UTf[:, h * HALF : (h + 1) * HALF],
                            start=True,
                            stop=True,
                        )
                    GZ = fin.tile([D + 1, CHW], F32, tag="GZ")
                    nc.vector.tensor_copy(GZ[0:D, :], G[:])
                    nc.vector.tensor_copy(GZ[D : D + 1, :], U[D : D + 1, :])
                    for t in range(CHW // P):
                        gi = a * (CHW // P) + t
                        T = ps_big.tile([P, D + 1], F32, tag="big")
                        nc.tensor.transpose(
                            T[:],
                            GZ[:, t * P : (t + 1) * P],
                            identf[0 : D + 1, 0 : D + 1],
                        )
                        rz = tmp.tile([P, 1], F32, tag="rz")
                        nc.vector.reciprocal(rz[:], T[:, D : D + 1])
                        tmpo = tmp.tile([P, D], F32, tag="tmpo")
                        nc.vector.tensor_scalar_mul(tmpo[:], T[:, 0:D], rz[:])
                        nc.vector.tensor_add(tmpo[:], tmpo[:], xb[p, blk][:, gi, :])
                        if not last:
                            dst = xb[p, blk + 1][:, gi, :]
                            nc.vector.tensor_scalar_max(dst, tmpo[:], 0.0)
                            nc.vector.tensor_copy(
                                xb16[p, blk + 1][:, gi, 0:D], dst
                            )
                        else:
                            oo = tmp.tile([P, D], F32, tag="oo")
                            nc.vector.tensor_scalar_max(oo[:], tmpo[:], 0.0)
                            nc.sync.dma_start(out_t[p][:, gi, :], oo[:])

            for blk in range(nblocks):
                for p in range(npairs):
                    prep(p, blk)
                process_both(blk, scales[blk], last=(blk == nblocks - 1))

    nc.compile()
    return nc


_CACHE = {}


def _get_nc(scales, n_rows, npairs):
    key = (tuple(scales), n_rows, npairs)
    if key not in _CACHE:
        _CACHE[key] = build_nc(list(scales), n_rows=n_rows, npairs=npairs)
    return _CACHE[key]


def kernel(x, W1, W2, alpha1, alpha2):
    x = np.asarray(x, dtype=np.float32)
    B, H, N, d = x.shape
    assert d == D and (B * H) % N_CORES == 0
    npairs = (B * H) // N_CORES
    s1 = 1.0 / max(float(alpha1), 0.01)
    s2 = 1.0 / max(float(alpha2), 0.01)
    nc = _get_nc((s1, s2), N, npairs)

    xf = np.ascontiguousarray(x.reshape(B * H, N, d))
    w0 = np.ascontiguousarray(np.asarray(W1, dtype=np.float32).T)
    w1 = np.ascontiguousarray(np.asarray(W2, dtype=np.float32).T)
    in_maps = [
        {"xin": xf[npairs * c : npairs * (c + 1)], "w0t": w0, "w1t": w1}
        for c in range(N_CORES)
    ]
    res = run_bass_kernel_spmd(nc, in_maps, core_ids=list(range(N_CORES)))
    outs = np.stack([r["out"] for r in res.results])
    return outs.reshape(B, H, N, d).astype(np.float32)



# revision 6
# speedup vs baseline: 1.2163x; 1.2163x over previous
"""Trainium2 Bass kernel for a 2-layer cosine-similarity attention GCN.

Reference math (per (b,h) slice, two chained blocks):
    xn = x / max(||x||_row, eps)
    A  = softmax((xn @ xn^T) / max(alpha, 0.01), axis=-1)
    out = relu((A @ x) @ W^T + x)

Shapes: x [4, 4, 4096, 64] fp32; W [64, 64]. B*H = 16 slices sharded as
2 slices per NeuronCore across 8 cores (fully independent, no collectives).

Kernel strategy (per core, 2 pairs x 2 blocks, all on-chip):
  - logits are cosine sims in [-1,1]*scale -> softmax without max-subtraction:
    P = exp(S*scale) / Z with Z = rowsum, obtained for free by a ones column
    in the stationary operand of the AV matmul.
  - W is fused into the AV matmul: lhsT = [1 | x@W^T | 0pad], so U = [Z; G]
    with G = W (x^T E) directly; no separate W matmul, no U copy.
  - E tiles are produced in [j, i] orientation (j on partitions) so E @ x
    contracts over partitions; the softmax matrix never exists in HBM.
  - exp is split between the scalar engine (ACT, true exp) and the vector
    engine (Schraudolph bit-trick exp -> bf16, 2 tensor_scalar ops) so the
    two engines share the N^2 exponential work.
  - the two pairs' S matmuls run in different PE row groups (partitions
    0-63 / 64-127) so they execute concurrently.
  - U ([80, CHW] psum) is copied to bf16 and DMA-xbar-transposed to
    row-major; division by Z, residual add and relu happen row-major where
    Z is a per-partition scalar.
"""

import numpy as np

import concourse.bacc as bacc
import concourse.tile as tile
from concourse import mybir
from concourse.bass_utils import run_bass_kernel_spmd
from concourse.masks import make_identity
from concourse.dve_ops import TENSOR_TENSOR_REDUCE

F32 = mybir.dt.float32
FP16 = mybir.dt.float16
I32 = mybir.dt.int32
U16 = mybir.dt.uint16
BF16 = mybir.dt.bfloat16
AF = mybir.ActivationFunctionType
ALU = mybir.AluOpType

P = 128
D = 64
N_CORES = 8
ACT_W = 768          # columns of each 1024-chunk exp'd on the scalar engine
MAGIC = 0x5F3759DF   # fast inverse sqrt seed


def _schraudolph_consts(scale):
    """Constants for the 2-op DVE exp producing fp16 bits directly:
    fp16bits(exp(scale*s)) ~= bits(s*scale*K16 + 1.5*2^23) + EADD, where the
    int add's i32->u16 writeback cast takes the low halfword."""
    K16 = 2.0 ** 10 / np.log(2.0)
    c_err = 44
    eadd = 15 * 1024 - c_err - 1262485504
    return float(scale * K16), float(1.5 * 2 ** 23), int(eadd)


def build_nc(scales, n_rows=4096, npairs=2):
    nblocks = len(scales)
    NT = n_rows // P             # 128-row tiles per pair
    CHW = 1024                   # i-chunk width
    NCH = n_rows // CHW
    HALF = 512                   # fp32 PSUM bank width (matmul free dim)
    NH = CHW // HALF
    DVE_W = CHW - ACT_W          # columns exp'd on the vector engine
    MW = 80                      # U partitions: [Z | G(64) | pad(15)]

    nc = bacc.Bacc("TRN2", target_bir_lowering=False, debug=False, num_devices=N_CORES)
    xin = nc.dram_tensor("xin", [npairs, n_rows, D], F32, kind="ExternalInput").ap()
    wts = [
        nc.dram_tensor(f"w{i}t", [D, D], F32, kind="ExternalInput").ap()
        for i in range(nblocks)
    ]
    out = nc.dram_tensor("out", [npairs, n_rows, D], F32, kind="ExternalOutput").ap()

    xin_t = xin.rearrange("p (t pp) d -> p pp t d", pp=P)  # [np, 128, NT, 64]
    out_t = out.rearrange("p (t pp) d -> p pp t d", pp=P)

    with tile.TileContext(nc) as tc:
        with (
            tc.tile_pool(name="singles", bufs=1) as singles,
            tc.tile_pool(name="stats", bufs=2) as stats,
            tc.tile_pool(name="tmp", bufs=3) as tmp,
            tc.tile_pool(name="epool", bufs=6) as epool,
            tc.tile_pool(name="dvex", bufs=4) as dvex,
            tc.tile_pool(name="fin", bufs=2) as fin,
            tc.tile_pool(name="ps_s", bufs=2, space="PSUM") as ps_s,
            tc.tile_pool(name="ps_u", bufs=2, space="PSUM") as ps_u,
        ):
            ident16 = singles.tile([P, P], BF16, tag="ident16")
            make_identity(nc, ident16[:])

            # W.T tiles, cast to bf16, replicated on both partition halves so
            # pair-1 xw matmuls (lhsT based at partition 64) see them too.
            wt16 = []
            for i in range(nblocks):
                wf = singles.tile([D, D], F32, tag=f"wf{i}", name=f"wf{i}")
                nc.sync.dma_start(wf[:], wts[i])
                w16 = singles.tile([P, D], BF16, tag=f"w16_{i}", name=f"w16_{i}")
                nc.vector.tensor_copy(w16[0:D, :], wf[:])
                nc.vector.tensor_copy(w16[D:P, :], wf[:])
                wt16.append(w16)

            # Persistent per (pair, block) state.
            xnt = {}   # normalized rows, transposed: pair p at partitions [64p, 64p+64)
            xb = {}    # block input, row-major fp32 [128, NT, 64]
            xw16 = {}  # U-matmul stationary operand: [1 | x@W^T | 0] bf16
            for blk in range(nblocks):
                xnt[blk] = singles.tile([P, n_rows], BF16, tag=f"xnt_{blk}", name=f"xnt_{blk}")
            for p in range(npairs):
                for blk in range(nblocks):
                    xb[p, blk] = singles.tile(
                        [P, NT, D], F32, tag=f"xb_{p}_{blk}", name=f"xb_{p}_{blk}"
                    )
                    xw16[p, blk] = singles.tile(
                        [P, NT, MW], FP16, tag=f"xw_{p}_{blk}", name=f"xw_{p}_{blk}"
                    )
                    nc.vector.memset(xw16[p, blk][:], 0.0)
                    nc.vector.memset(xw16[p, blk][:, :, 0:1], 1.0)

            # norms^2 accumulators (block-1's filled during block-0 epilogue)
            s_all = {
                (p, blk): singles.tile([P, NT], F32, tag=f"sall_{p}_{blk}", name=f"sall_{p}_{blk}")
                for p in range(npairs)
                for blk in range(nblocks)
            }

            for p in range(npairs):
                nc.sync.dma_start(xb[p, 0][:], xin_t[p])

            def rsqrt_inplace(s_t, r_t, n):
                """r = s^-0.5 (fast inverse sqrt + 3 Newton steps); s clamped."""
                nc.vector.tensor_scalar_max(s_t[:], s_t[:], 1e-24)
                s_i = s_t[:].bitcast(I32)
                r_i = r_t[:].bitcast(I32)
                nc.vector.tensor_scalar(
                    out=r_i, in0=s_i, scalar1=1, scalar2=None,
                    op0=ALU.logical_shift_right,
                )
                nc.vector.tensor_scalar(
                    out=r_i, in0=r_i, scalar1=MAGIC, scalar2=None, op0=ALU.subtract,
                )
                nc.vector.tensor_scalar(
                    out=r_i, in0=r_i, scalar1=-1, scalar2=None, op0=ALU.bitwise_xor,
                )
                nc.vector.tensor_scalar(
                    out=r_i, in0=r_i, scalar1=1, scalar2=None, op0=ALU.add,
                )
                t1 = stats.tile([P, n], F32, tag="nt1")
                for _ in range(3):
                    nc.vector.tensor_mul(t1[:], r_t[:], r_t[:])
                    nc.vector.tensor_mul(t1[:], t1[:], s_t[:])
                    nc.vector.tensor_scalar(
                        out=t1[:], in0=t1[:], scalar1=-0.5, scalar2=1.5,
                        op0=ALU.mult, op1=ALU.add,
                    )
                    nc.vector.tensor_mul(r_t[:], r_t[:], t1[:])

            def prep(p, blk, norms_on_scalar):
                """rinv -> normalized bf16 rows -> PE-transpose into xnt;
                xw16 = norm * (xn @ W^T) via matmul + scale."""
                if norms_on_scalar:
                    sq = tmp.tile([P, D], F32, tag="sq")
                    for b in range(NT):
                        nc.scalar.activation(
                            sq[:], xb[p, blk][:, b, :], AF.Square,
                            accum_out=s_all[p, blk][:, b : b + 1],
                        )
                r = stats.tile([P, NT], F32, tag="rinv")
                rsqrt_inplace(s_all[p, blk], r, NT)
                nrm = stats.tile([P, NT], F32, tag="nrm")
                nc.vector.tensor_mul(nrm[:], s_all[p, blk][:], r[:])
                lo = D * p
                for b in range(NT):
                    xn16 = tmp.tile([P, D], BF16, tag="xn16")
                    nc.vector.tensor_scalar_mul(
                        xn16[:], xb[p, blk][:, b, :], r[:, b : b + 1]
                    )
                    pst = ps_s.tile([P, P], BF16, tag="S")
                    nc.tensor.transpose(pst[lo : lo + D, :], xn16[:], ident16[:])
                    nc.vector.tensor_copy(
                        xnt[blk][lo : lo + D, b * P : (b + 1) * P],
                        pst[lo : lo + D, :],
                    )
                for b in range(NT):
                    psw = ps_u.tile([P, D], F32, tag="U")
                    nc.tensor.matmul(
                        psw[:],
                        lhsT=xnt[blk][lo : lo + D, b * P : (b + 1) * P],
                        rhs=wt16[blk][lo : lo + D, :],
                        start=True, stop=True,
                    )
                    nc.vector.tensor_scalar_mul(
                        xw16[p, blk][:, b, 1 : 1 + D], psw[:], nrm[:, b : b + 1]
                    )

            def process(blk, scale, last):
                smul, smagic, eadd = _schraudolph_consts(scale)
                for a in range(NCH):
                    U = {}
                    for p in range(npairs):
                        U[p] = ps_u.tile([MW, CHW], F32, tag="U", name=f"U_{blk}_{a}_{p}")

                    def u_mms(p, b, E_b):
                        for h in range(NH):
                            nc.tensor.matmul(
                                U[p][:, h * HALF : (h + 1) * HALF],
                                lhsT=xw16[p, blk][:, b, :],
                                rhs=E_b[:, h * HALF : (h + 1) * HALF],
                                start=(b == 0),
                                stop=(b == NT - 1),
                            )

                    E_prev = None
                    for b in range(NT):
                        S = {}
                        for p in range(npairs):
                            lo = D * p
                            xnt_p = xnt[blk][lo : lo + D, :]
                            S[p] = ps_s.tile([P, CHW], F32, tag="S", name=f"S_{b}_{p}")
                            for h in range(NH):
                                nc.tensor.matmul(
                                    S[p][:, h * HALF : (h + 1) * HALF],
                                    lhsT=xnt_p[:, b * P : (b + 1) * P],
                                    rhs=xnt_p[
                                        :, a * CHW + h * HALF : a * CHW + (h + 1) * HALF
                                    ],
                                    start=True, stop=True,
                                )
                        E = {}
                        for p in range(npairs):
                            E[p] = epool.tile([P, CHW], FP16, tag="E", name=f"E_{b}_{p}")
                            # scalar engine: true exp on the first ACT_W cols
                            nc.scalar.activation(
                                E[p][:, 0:ACT_W], S[p][:, 0:ACT_W], AF.Exp, scale=scale
                            )
                            # vector engine: Schraudolph exp -> bf16 bits
                            ft = dvex.tile([P, DVE_W], F32, tag="ft")
                            nc.vector.tensor_scalar(
                                out=ft[:], in0=S[p][:, ACT_W:CHW],
                                scalar1=smul, scalar2=smagic,
                                op0=ALU.mult, op1=ALU.add,
                            )
                            nc.vector.tensor_scalar(
                                out=E[p][:, ACT_W:CHW].bitcast(U16),
                                in0=ft[:].bitcast(I32),
                                scalar1=eadd, scalar2=None, op0=ALU.add,
                            )
                        if E_prev is not None:
                            for p in range(npairs):
                                u_mms(p, b - 1, E_prev[p])
                        E_prev = E
                    for p in range(npairs):
                        u_mms(p, NT - 1, E_prev[p])

                    # chunk epilogue per pair: bf16 copy of U, xbar-transpose to
                    # row-major [128, 8, 80], then out = relu(G/Z + x) row-major.
                    for p in range(npairs):
                        u16 = fin.tile([MW, CHW], BF16, tag="u16")
                        nc.vector.tensor_copy(u16[:], U[p][:])
                        T = fin.tile([P, CHW // P, MW], BF16, tag="T")
                        nc.sync.dma_start_transpose(T[:], u16[:])
                        rz = tmp.tile([P, CHW // P], F32, tag="rz")
                        nc.vector.reciprocal(rz[:], T[:, :, 0])
                        gm = fin.tile([P, CHW // P, D], F32, tag="gm")
                        for t in range(CHW // P):
                            nc.vector.tensor_scalar_mul(
                                gm[:, t, :], T[:, t, 1 : 1 + D], rz[:, t : t + 1]
                            )
                        gi0 = a * (CHW // P)
                        gin = CHW // P
                        nc.vector.tensor_add(
                            gm[:], gm[:], xb[p, blk][:, gi0 : gi0 + gin, :]
                        )
                        if not last:
                            dst = xb[p, blk + 1][:, gi0 : gi0 + gin, :]
                            nc.vector.tensor_scalar_max(dst, gm[:], 0.0)
                            # norms^2 of the next block's rows (one fused op per tile)
                            sqo = tmp.tile([P, D], F32, tag="sqo")
                            for t in range(gin):
                                nc.vector._custom_dve(
                                    TENSOR_TENSOR_REDUCE,
                                    out=sqo[:],
                                    in0=xb[p, blk + 1][:, gi0 + t, :],
                                    in1=xb[p, blk + 1][:, gi0 + t, :],
                                    s0=0.0, s1=1.0,
                                    accum_out=s_all[p, blk + 1][:, gi0 + t : gi0 + t + 1],
                                )
                        else:
                            oo = fin.tile([P, CHW // P, D], F32, tag="oo")
                            nc.vector.tensor_scalar_max(oo[:], gm[:], 0.0)
                            nc.sync.dma_start(
                                out_t[p][:, gi0 : gi0 + gin, :], oo[:]
                            )

            for p in range(npairs):
                prep(p, 0, norms_on_scalar=True)
            for blk in range(nblocks):
                process(blk, scales[blk], last=(blk == nblocks - 1))
                if blk + 1 < nblocks:
                    for p in range(npairs):
                        prep(p, blk + 1, norms_on_scalar=False)

    nc.compile()
    return nc


_CACHE = {}


def _get_nc(scales, n_rows, npairs):
    key = (tuple(scales), n_rows, npairs)
    if key not in _CACHE:
        _CACHE[key] = build_nc(list(scales), n_rows=n_rows, npairs=npairs)
    return _CACHE[key]


def kernel(x, W1, W2, alpha1, alpha2):
    x = np.asarray(x, dtype=np.float32)
    B, H, N, d = x.shape
    assert d == D and (B * H) % N_CORES == 0
    npairs = (B * H) // N_CORES
    s1 = 1.0 / max(float(alpha1), 0.01)
    s2 = 1.0 / max(float(alpha2), 0.01)
    nc = _get_nc((s1, s2), N, npairs)

    xf = np.ascontiguousarray(x.reshape(B * H, N, d))
    w0 = np.ascontiguousarray(np.asarray(W1, dtype=np.float32).T)
    w1 = np.ascontiguousarray(np.asarray(W2, dtype=np.float32).T)
    in_maps = [
        {"xin": xf[npairs * c : npairs * (c + 1)], "w0t": w0, "w1t": w1}
        for c in range(N_CORES)
    ]
    res = run_bass_kernel_spmd(nc, in_maps, core_ids=list(range(N_CORES)))
    outs = np.stack([r["out"] for r in res.results])
    return outs.reshape(B, H, N, d).astype(np.float32)


# revision 11
# speedup vs baseline: 1.3922x; 1.1446x over previous
"""Trainium2 Bass kernel for a 2-layer cosine-similarity attention GCN.

Reference math (per (b,h) slice, two chained blocks):
    xn = x / max(||x||_row, eps)
    A  = softmax((xn @ xn^T) / max(alpha, 0.01), axis=-1)
    out = relu((A @ x) @ W^T + x)

Shapes: x [4, 4, 4096, 64] fp32; W [64, 64]. B*H = 16 slices sharded as
2 slices per NeuronCore across 8 cores (fully independent, no collectives).

Kernel strategy (per core, 2 pairs x 2 blocks, all on-chip):
  - softmax without max-subtraction (logits are bounded cosine sims):
    P = exp(S*scale)/Z, Z = rowsum via a ones column in the AV matmul.
  - W fused into the AV matmul: lhsT = [1 | x@W^T | 0pad] so U = [Z; G].
  - exp split between the scalar engine (true exp) and the vector engine
    (Schraudolph bit-trick exp emitted directly as fp16 bits, 2 ops).
  - flattened software-pipelined step loop over all (chunk, tile) steps:
    S matmuls (N=1024, the two pairs in different PE row groups), exp, AV
    matmuls at lag 2; chunk epilogues and next-block prep are sliced into
    small deferred pieces drained a few per step so no engine ever sees a
    multi-us bubble (keeps the PE HAM clock-gate at 8/8).
  - U ([80, CHW] psum) -> bf16 -> DMA-xbar transpose to row-major; divide
    by Z / residual / relu done row-major where Z is a per-partition scalar.
"""

import numpy as np

import concourse.bacc as bacc
import concourse.tile as tile
from concourse import mybir
from concourse.bass_utils import run_bass_kernel_spmd
from concourse.masks import make_identity
from concourse.dve_ops import TENSOR_TENSOR_REDUCE

F32 = mybir.dt.float32
FP16 = mybir.dt.float16
I32 = mybir.dt.int32
U16 = mybir.dt.uint16
BF16 = mybir.dt.bfloat16
AF = mybir.ActivationFunctionType
ALU = mybir.AluOpType

P = 128
D = 64
N_CORES = 8
ACT_W = 736          # columns of each 1024-chunk exp'd on the scalar engine
MAGIC = 0x5F3759DF   # fast inverse sqrt seed


def _schraudolph_consts(scale):
    """Constants for the 2-op DVE exp producing fp16 bits directly:
    fp16bits(exp(scale*s)) ~= bits(s*scale*K16 + 1.5*2^23) + EADD, where the
    int add's i32->u16 writeback cast takes the low halfword."""
    K16 = 2.0 ** 10 / np.log(2.0)
    c_err = 44
    eadd = 15 * 1024 - c_err - 1262485504
    return float(scale * K16), float(1.5 * 2 ** 23), int(eadd)


def build_nc(scales, n_rows=4096, npairs=2):
    nblocks = len(scales)
    NT = n_rows // P             # 128-row tiles per pair
    CHW = 1024                   # i-chunk width
    NCH = n_rows // CHW
    GPC = CHW // P               # row-major gi tiles per chunk (8)
    DVE_W = CHW - ACT_W
    MW = 80                      # U partitions: [Z | G(64) | pad(15)]
    LAG = 2                      # steps between S/exp production and AV use

    nc = bacc.Bacc("TRN2", target_bir_lowering=False, debug=False, num_devices=N_CORES)
    xin = nc.dram_tensor("xin", [npairs, n_rows, D], F32, kind="ExternalInput").ap()
    wts = [
        nc.dram_tensor(f"w{i}t", [D, D], F32, kind="ExternalInput").ap()
        for i in range(nblocks)
    ]
    out = nc.dram_tensor("out", [npairs, n_rows, D], F32, kind="ExternalOutput").ap()

    xin_t = xin.rearrange("p (t pp) d -> p pp t d", pp=P)  # [np, 128, NT, 64]
    out_t = out.rearrange("p (t pp) d -> p pp t d", pp=P)

    with tile.TileContext(nc) as tc:
        with (
            tc.tile_pool(name="singles", bufs=1) as singles,
            tc.tile_pool(name="stats", bufs=2) as stats,
            tc.tile_pool(name="tmp", bufs=3) as tmp,
            tc.tile_pool(name="epool", bufs=6) as epool,
            tc.tile_pool(name="dvex", bufs=4) as dvex,
            tc.tile_pool(name="fin", bufs=2) as fin,
            tc.tile_pool(name="ps_s", bufs=2, space="PSUM") as ps_s,
            tc.tile_pool(name="ps_u", bufs=2, space="PSUM") as ps_u,
        ):
            ident16 = singles.tile([P, P], BF16, tag="ident16")
            make_identity(nc, ident16[:])

            # W.T tiles, cast to bf16, replicated on both partition halves so
            # pair-1 xw matmuls (lhsT based at partition 64) see them too.
            wt16 = []
            for i in range(nblocks):
                wf = singles.tile([D, D], F32, tag=f"wf{i}", name=f"wf{i}")
                nc.sync.dma_start(wf[:], wts[i])
                w16 = singles.tile([P, D], BF16, tag=f"w16_{i}", name=f"w16_{i}")
                nc.vector.tensor_copy(w16[0:D, :], wf[:])
                nc.vector.tensor_copy(w16[D:P, :], wf[:])
                wt16.append(w16)

            # Persistent per (pair, block) state.
            xnt = {}   # normalized rows, transposed: pair p at partitions [64p, 64p+64)
            xb = {}    # block input, row-major fp32 [128, NT, 64]
            xw16 = {}  # AV-matmul stationary operand: [1 | x@W^T | 0] fp16
            for blk in range(nblocks):
                xnt[blk] = singles.tile([P, n_rows], BF16, tag=f"xnt_{blk}", name=f"xnt_{blk}")
            for p in range(npairs):
                for blk in range(nblocks):
                    xb[p, blk] = singles.tile(
                        [P, NT, D], F32, tag=f"xb_{p}_{blk}", name=f"xb_{p}_{blk}"
                    )
                    xw16[p, blk] = singles.tile(
                        [P, NT, MW], FP16, tag=f"xw_{p}_{blk}", name=f"xw_{p}_{blk}"
                    )
                    nc.vector.memset(xw16[p, blk][:], 0.0)
                    nc.vector.memset(xw16[p, blk][:, :, 0:1], 1.0)

            # norms^2 / 1/norm / norm accumulators per (pair, block)
            s_all = {}
            rin_all = {}
            nrm_all = {}
            for p in range(npairs):
                for blk in range(nblocks):
                    s_all[p, blk] = singles.tile(
                        [P, NT], F32, tag=f"sall_{p}_{blk}", name=f"sall_{p}_{blk}"
                    )
                    rin_all[p, blk] = singles.tile(
                        [P, NT], F32, tag=f"rin_{p}_{blk}", name=f"rin_{p}_{blk}"
                    )
                    nrm_all[p, blk] = singles.tile(
                        [P, NT], F32, tag=f"nrm_{p}_{blk}", name=f"nrm_{p}_{blk}"
                    )

            for p in range(npairs):
                nc.sync.dma_start(xb[p, 0][:], xin_t[p])

            def rsqrt_slice(p, blk, c0, n):
                """rin = s^-0.5, nrm = s*rin for columns [c0, c0+n)."""
                s_t = s_all[p, blk][:, c0 : c0 + n]
                r_t = rin_all[p, blk][:, c0 : c0 + n]
                nc.vector.tensor_scalar_max(s_t, s_t, 1e-24)
                s_i = s_t.bitcast(I32)
                r_i = r_t.bitcast(I32)
                nc.vector.tensor_scalar(
                    out=r_i, in0=s_i, scalar1=1, scalar2=None,
                    op0=ALU.logical_shift_right,
                )
                nc.vector.tensor_scalar(
                    out=r_i, in0=r_i, scalar1=MAGIC, scalar2=None, op0=ALU.subtract,
                )
                nc.vector.tensor_scalar(
                    out=r_i, in0=r_i, scalar1=-1, scalar2=None, op0=ALU.bitwise_xor,
                )
                nc.vector.tensor_scalar(
                    out=r_i, in0=r_i, scalar1=1, scalar2=None, op0=ALU.add,
                )
                t1 = stats.tile([P, NT], F32, tag="nt1")
                t1v = t1[:, 0:n]
                for _ in range(3):
                    nc.vector.tensor_mul(t1v, r_t, r_t)
                    nc.vector.tensor_mul(t1v, t1v, s_t)
                    nc.vector.tensor_scalar(
                        out=t1v, in0=t1v, scalar1=-0.5, scalar2=1.5,
                        op0=ALU.mult, op1=ALU.add,
                    )
                    nc.vector.tensor_mul(r_t, r_t, t1v)
                nc.vector.tensor_mul(nrm_all[p, blk][:, c0 : c0 + n], s_t, r_t)

            def xn_tile(p, blk, b):
                """normalize tile b, PE-transpose it into xnt."""
                lo = D * p
                xn16 = tmp.tile([P, D], BF16, tag="xn16")
                nc.vector.tensor_scalar_mul(
                    xn16[:], xb[p, blk][:, b, :], rin_all[p, blk][:, b : b + 1]
                )
                pst = ps_s.tile([P, P], BF16, tag="S")
                nc.tensor.transpose(pst[lo : lo + D, :], xn16[:], ident16[:])
                nc.vector.tensor_copy(
                    xnt[blk][lo : lo + D, b * P : (b + 1) * P], pst[lo : lo + D, :]
                )

            def xw_tile(p, blk, b):
                """xw16[:, b, 1:65] = norm_b * (xn_b @ W^T) via matmul + scale."""
                lo = D * p
                psw = ps_u.tile([P, D], F32, tag="U")
                nc.tensor.matmul(
                    psw[:],
                    lhsT=xnt[blk][lo : lo + D, b * P : (b + 1) * P],
                    rhs=wt16[blk][lo : lo + D, :],
                    start=True, stop=True,
                )
                nc.vector.tensor_scalar_mul(
                    xw16[p, blk][:, b, 1 : 1 + D], psw[:], nrm_all[p, blk][:, b : b + 1]
                )

            def prep0(p):
                """Full upfront prep for block 0 (norms on the scalar engine)."""
                sq = tmp.tile([P, D], F32, tag="sq")
                for b in range(NT):
                    nc.scalar.activation(
                        sq[:], xb[p, 0][:, b, :], AF.Square,
                        accum_out=s_all[p, 0][:, b : b + 1],
                    )
                rsqrt_slice(p, 0, 0, NT)
                for b in range(NT):
                    xn_tile(p, 0, b)
                for b in range(NT):
                    xw_tile(p, 0, b)

            def process(blk, scale, last):
                smul, smagic, eadd = _schraudolph_consts(scale)
                nsteps = NCH * NT
                U = {}        # chunk -> pair -> psum tile
                E_hist = {}   # step -> pair -> E tile
                deferred = []

                def u_mms(k):
                    a_, b_ = divmod(k, NT)
                    for p in range(npairs):
                        for h in range(2):
                            nc.tensor.matmul(
                                U[a_][p][:, h * 512 : (h + 1) * 512],
                                lhsT=xw16[p, blk][:, b_, :],
                                rhs=E_hist[k][p][:, h * 512 : (h + 1) * 512],
                                start=(b_ == 0),
                                stop=(b_ == NT - 1),
                            )
                    if k in E_hist:
                        del E_hist[k]

                def make_epilogue_pieces(a):
                    """Small deferred closures; drained a couple per step."""
                    pieces = []
                    gi0 = a * GPC
                    u16 = {}
                    T = {}
                    rz = {}
                    gm = {}

                    def mk(p):
                        def c_copy_l():
                            u16[p] = fin.tile([MW, CHW], BF16, tag="u16", name=f"u16_{blk}_{a}_{p}")
                            nc.vector.tensor_copy(u16[p][:, 0:512], U[a][p][:, 0:512])

                        def c_copy_r():
                            nc.vector.tensor_copy(u16[p][:, 512:1024], U[a][p][:, 512:1024])

                        def c_tr():
                            T[p] = fin.tile([P, GPC, MW], BF16, tag="T", name=f"T_{blk}_{a}_{p}")
                            nc.sync.dma_start_transpose(T[p][:], u16[p][:])
                            rz[p] = tmp.tile([P, GPC], F32, tag="rz", name=f"rz_{blk}_{a}_{p}")
                            nc.vector.reciprocal(rz[p][:], T[p][:, :, 0])

                        def c_mul_lo():
                            gm[p] = fin.tile([P, GPC, D], F32, tag="gm", name=f"gm_{blk}_{a}_{p}")
                            for t in range(GPC // 2):
                                nc.vector.tensor_scalar_mul(
                                    gm[p][:, t, :], T[p][:, t, 1 : 1 + D],
                                    rz[p][:, t : t + 1],
                                )

                        def c_mul_hi():
                            for t in range(GPC // 2, GPC):
                                nc.vector.tensor_scalar_mul(
                                    gm[p][:, t, :], T[p][:, t, 1 : 1 + D],
                                    rz[p][:, t : t + 1],
                                )

                        def c_add():
                            nc.vector.tensor_add(
                                gm[p][:], gm[p][:], xb[p, blk][:, gi0 : gi0 + GPC, :]
                            )

                        def c_relu():
                            if not last:
                                dst = xb[p, blk + 1][:, gi0 : gi0 + GPC, :]
                                nc.vector.tensor_scalar_max(dst, gm[p][:], 0.0)
                            else:
                                oo = fin.tile([P, GPC, D], F32, tag="oo", name=f"oo_{blk}_{a}_{p}")
                                nc.vector.tensor_scalar_max(oo[:], gm[p][:], 0.0)
                                nc.sync.dma_start(out_t[p][:, gi0 : gi0 + GPC, :], oo[:])

                        return [c_copy_l, c_copy_r, c_tr, c_mul_lo, c_mul_hi, c_add, c_relu]

                    per_p = [mk(p) for p in range(npairs)]
                    for idx in range(len(per_p[0])):
                        for p in range(npairs):
                            pieces.append(per_p[p][idx])

                    if not last:
                        # next block's norms + rsqrt + xnt tiles for this chunk
                        def mk_prep(p):
                            sub = []

                            def c_norms_lo():
                                sqo = tmp.tile([P, D], F32, tag="sqo")
                                for t in range(GPC // 2):
                                    gi = gi0 + t
                                    nc.vector._custom_dve(
                                        TENSOR_TENSOR_REDUCE,
                                        out=sqo[:],
                                        in0=xb[p, blk + 1][:, gi, :],
                                        in1=xb[p, blk + 1][:, gi, :],
                                        s0=0.0, s1=1.0,
                                        accum_out=s_all[p, blk + 1][:, gi : gi + 1],
                                    )

                            def c_norms_hi():
                                sqo = tmp.tile([P, D], F32, tag="sqo")
                                for t in range(GPC // 2, GPC):
                                    gi = gi0 + t
                                    nc.vector._custom_dve(
                                        TENSOR_TENSOR_REDUCE,
                                        out=sqo[:],
                                        in0=xb[p, blk + 1][:, gi, :],
                                        in1=xb[p, blk + 1][:, gi, :],
                                        s0=0.0, s1=1.0,
                                        accum_out=s_all[p, blk + 1][:, gi : gi + 1],
                                    )

                            def c_rsqrt():
                                rsqrt_slice(p, blk + 1, gi0, GPC)

                            sub.extend([c_norms_lo, c_norms_hi, c_rsqrt])
                            for t in range(GPC):
                                sub.append(
                                    (lambda tt: lambda: xn_tile(p, blk + 1, gi0 + tt))(t)
                                )
                            return sub

                        per_pp = [mk_prep(p) for p in range(npairs)]
                        for idx in range(len(per_pp[0])):
                            for p in range(npairs):
                                pieces.append(per_pp[p][idx])
                    return pieces

                for k in range(nsteps):
                    a, b = divmod(k, NT)
                    if b == 0:
                        U[a] = {
                            p: ps_u.tile([MW, CHW], F32, tag="U", name=f"U_{blk}_{a}_{p}")
                            for p in range(npairs)
                        }
                    # S matmuls, both pairs adjacent (different PE row groups)
                    S = {}
                    for p in range(npairs):
                        S[p] = ps_s.tile([P, CHW], F32, tag="S", name=f"S_{k}_{p}")
                    for h in range(2):
                        for p in range(npairs):
                            lo = D * p
                            nc.tensor.matmul(
                                S[p][:, h * 512 : (h + 1) * 512],
                                lhsT=xnt[blk][lo : lo + D, b * P : (b + 1) * P],
                                rhs=xnt[blk][
                                    lo : lo + D, a * CHW + h * 512 : a * CHW + (h + 1) * 512
                                ],
                                start=True, stop=True,
                            )
                    # exp: scalar engine + vector engine split
                    E_hist[k] = {}
                    for p in range(npairs):
                        Ek = epool.tile([P, CHW], FP16, tag="E", name=f"E_{k}_{p}")
                        E_hist[k][p] = Ek
                        nc.scalar.activation(
                            Ek[:, 0:ACT_W], S[p][:, 0:ACT_W], AF.Exp, scale=scale
                        )
                        ft = dvex.tile([P, DVE_W], F32, tag="ft")
                        nc.vector.tensor_scalar(
                            out=ft[:], in0=S[p][:, ACT_W:CHW],
                            scalar1=smul, scalar2=smagic,
                            op0=ALU.mult, op1=ALU.add,
                        )
                        nc.vector.tensor_scalar(
                            out=Ek[:, ACT_W:CHW].bitcast(U16),
                            in0=ft[:].bitcast(I32),
                            scalar1=eadd, scalar2=None, op0=ALU.add,
                        )
                    # AV matmuls at lag
                    if k >= LAG:
                        u_mms(k - LAG)
                        kk = k - LAG
                        if kk % NT == NT - 1:
                            deferred.extend(make_epilogue_pieces(kk // NT))
                    # drain deferred pieces
                    for _ in range(2):
                        if deferred:
                            deferred.pop(0)()

                # tail
                for k in range(nsteps - LAG, nsteps):
                    u_mms(k)
                deferred.extend(make_epilogue_pieces(NCH - 1))
                while deferred:
                    deferred.pop(0)()
                if not last:
                    # xw matmuls for the next block (PSUM rings are free here)
                    for b in range(NT):
                        for p in range(npairs):
                            xw_tile(p, blk + 1, b)

            for p in range(npairs):
                prep0(p)
            for blk in range(nblocks):
                process(blk, scales[blk], last=(blk == nblocks - 1))

    nc.compile()
    return nc


_CACHE = {}


def _get_nc(scales, n_rows, npairs):
    key = (tuple(scales), n_rows, npairs)
    if key not in _CACHE:
        _CACHE[key] = build_nc(list(scales), n_rows=n_rows, npairs=npairs)
    return _CACHE[key]


def kernel(x, W1, W2, alpha1, alpha2):
    x = np.asarray(x, dtype=np.float32)
    B, H, N, d = x.shape
    assert d == D and (B * H) % N_CORES == 0
    npairs = (B * H) // N_CORES
    s1 = 1.0 / max(float(alpha1), 0.01)
    s2 = 1.0 / max(float(alpha2), 0.01)
    nc = _get_nc((s1, s2), N, npairs)

    xf = np.ascontiguousarray(x.reshape(B * H, N, d))
    w0 = np.ascontiguousarray(np.asarray(W1, dtype=np.float32).T)
    w1 = np.ascontiguousarray(np.asarray(W2, dtype=np.float32).T)
    in_maps = [
        {"xin": xf[npairs * c : npairs * (c + 1)], "w0t": w0, "w1t": w1}
        for c in range(N_CORES)
    ]
    res = run_bass_kernel_spmd(nc, in_maps, core_ids=list(range(N_CORES)))
    outs = np.stack([r["out"] for r in res.results])
    return outs.reshape(B, H, N, d).astype(np.float32)


# revision 12
# speedup vs baseline: 1.6284x; 1.1696x over previous
"""Trainium2 Bass kernel for a 2-layer cosine-similarity attention GCN.

Reference math (per (b,h) slice, two chained blocks):
    xn = x / max(||x||_row, eps)
    A  = softmax((xn @ xn^T) / max(alpha, 0.01), axis=-1)
    out = relu((A @ x) @ W^T + x)

Shapes: x [4, 4, 4096, 64] fp32; W [64, 64]. B*H = 16 slices sharded as
2 slices per NeuronCore across 8 cores (fully independent, no collectives).

Kernel strategy (per core, 2 pairs x 2 blocks, all on-chip):
  - softmax without max-subtraction (logits are bounded cosine sims):
    P = exp(S*scale)/Z, Z = rowsum via a ones column in the AV matmul.
  - W fused into the AV matmul: lhsT = [1 | x@W^T | 0pad] so U = [Z; G].
  - exp split between the scalar engine (true exp) and the vector engine
    (Schraudolph bit-trick exp emitted directly as fp16 bits, 2 ops).
  - flattened software-pipelined step loop over all (chunk, tile) steps:
    S matmuls (N=1024, the two pairs in different PE row groups), exp, AV
    matmuls at lag 2; chunk epilogues and next-block prep are sliced into
    small deferred pieces drained a few per step so no engine ever sees a
    multi-us bubble (keeps the PE HAM clock-gate at 8/8).
  - U ([80, CHW] psum) -> bf16 -> DMA-xbar transpose to row-major; divide
    by Z / residual / relu done row-major where Z is a per-partition scalar.
"""

import numpy as np

import concourse.bacc as bacc
import concourse.tile as tile
from concourse import mybir
from concourse.bass_utils import run_bass_kernel_spmd
from concourse.masks import make_identity
from concourse.dve_ops import TENSOR_TENSOR_REDUCE

F32 = mybir.dt.float32
FP16 = mybir.dt.float16
I32 = mybir.dt.int32
U16 = mybir.dt.uint16
BF16 = mybir.dt.bfloat16
AF = mybir.ActivationFunctionType
ALU = mybir.AluOpType

P = 128
D = 64
N_CORES = 8
ACT_W = 640          # columns of each 1024-chunk exp'd on the scalar engine
MAGIC = 0x5F3759DF   # fast inverse sqrt seed


def _schraudolph_consts(scale):
    """Constants for the 1-op DVE exp producing fp16 bits directly:
    fp16bits(exp(scale*s)) ~= u16(round(s*scale*K16 + BIAS)) -- the arith
    tensor_scalar's fp32->u16 writeback conversion does the float->int."""
    K16 = 2.0 ** 10 / np.log(2.0)
    c_err = 44
    return float(scale * K16), float(15 * 1024 - c_err)


def build_nc(scales, n_rows=4096, npairs=2):
    nblocks = len(scales)
    NT = n_rows // P             # 128-row tiles per pair
    CHW = 1024                   # i-chunk width
    NCH = n_rows // CHW
    GPC = CHW // P               # row-major gi tiles per chunk (8)
    DVE_W = CHW - ACT_W
    MW = 80                      # U partitions: [Z | G(64) | pad(15)]
    LAG = 2                      # steps between S/exp production and AV use

    nc = bacc.Bacc("TRN2", target_bir_lowering=False, debug=False, num_devices=N_CORES)
    xin = nc.dram_tensor("xin", [npairs, n_rows, D], F32, kind="ExternalInput").ap()
    wts = [
        nc.dram_tensor(f"w{i}t", [D, D], F32, kind="ExternalInput").ap()
        for i in range(nblocks)
    ]
    out = nc.dram_tensor("out", [npairs, n_rows, D], F32, kind="ExternalOutput").ap()

    xin_t = xin.rearrange("p (t pp) d -> p pp t d", pp=P)  # [np, 128, NT, 64]
    out_t = out.rearrange("p (t pp) d -> p pp t d", pp=P)

    with tile.TileContext(nc) as tc:
        with (
            tc.tile_pool(name="singles", bufs=1) as singles,
            tc.tile_pool(name="stats", bufs=2) as stats,
            tc.tile_pool(name="tmp", bufs=3) as tmp,
            tc.tile_pool(name="epool", bufs=6) as epool,
            tc.tile_pool(name="fin", bufs=2) as fin,
            tc.tile_pool(name="ps_s", bufs=2, space="PSUM") as ps_s,
            tc.tile_pool(name="ps_u", bufs=2, space="PSUM") as ps_u,
        ):
            ident16 = singles.tile([P, P], BF16, tag="ident16")
            make_identity(nc, ident16[:])

            # W.T tiles, cast to bf16, replicated on both partition halves so
            # pair-1 xw matmuls (lhsT based at partition 64) see them too.
            wt16 = []
            for i in range(nblocks):
                wf = singles.tile([D, D], F32, tag=f"wf{i}", name=f"wf{i}")
                nc.sync.dma_start(wf[:], wts[i])
                w16 = singles.tile([P, D], BF16, tag=f"w16_{i}", name=f"w16_{i}")
                nc.vector.tensor_copy(w16[0:D, :], wf[:])
                nc.vector.tensor_copy(w16[D:P, :], wf[:])
                wt16.append(w16)

            # Persistent per (pair, block) state.
            xnt = {}   # normalized rows, transposed: pair p at partitions [64p, 64p+64)
            xb = {}    # block input, row-major fp32 [128, NT, 64]
            xw16 = {}  # AV-matmul stationary operand: [1 | x@W^T | 0] fp16
            for blk in range(nblocks):
                xnt[blk] = singles.tile([P, n_rows], BF16, tag=f"xnt_{blk}", name=f"xnt_{blk}")
            for p in range(npairs):
                for blk in range(nblocks):
                    xb[p, blk] = singles.tile(
                        [P, NT, D], F32, tag=f"xb_{p}_{blk}", name=f"xb_{p}_{blk}"
                    )
                    xw16[p, blk] = singles.tile(
                        [P, NT, MW], FP16, tag=f"xw_{p}_{blk}", name=f"xw_{p}_{blk}"
                    )
                    nc.vector.memset(xw16[p, blk][:], 0.0)
                    nc.vector.memset(xw16[p, blk][:, :, 0:1], 1.0)

            # norms^2 / 1/norm / norm accumulators per (pair, block)
            s_all = {}
            rin_all = {}
            nrm_all = {}
            for p in range(npairs):
                for blk in range(nblocks):
                    s_all[p, blk] = singles.tile(
                        [P, NT], F32, tag=f"sall_{p}_{blk}", name=f"sall_{p}_{blk}"
                    )
                    rin_all[p, blk] = singles.tile(
                        [P, NT], F32, tag=f"rin_{p}_{blk}", name=f"rin_{p}_{blk}"
                    )
                    nrm_all[p, blk] = singles.tile(
                        [P, NT], F32, tag=f"nrm_{p}_{blk}", name=f"nrm_{p}_{blk}"
                    )

            for p in range(npairs):
                for c in range(NCH):
                    g0 = c * (NT // NCH)
                    g1 = (c + 1) * (NT // NCH)
                    nc.sync.dma_start(xb[p, 0][:, g0:g1, :], xin_t[p][:, g0:g1, :])

            def rsqrt_slice(p, blk, c0, n):
                """rin = s^-0.5, nrm = s*rin for columns [c0, c0+n)."""
                s_t = s_all[p, blk][:, c0 : c0 + n]
                r_t = rin_all[p, blk][:, c0 : c0 + n]
                nc.vector.tensor_scalar_max(s_t, s_t, 1e-24)
                s_i = s_t.bitcast(I32)
                r_i = r_t.bitcast(I32)
                nc.vector.tensor_scalar(
                    out=r_i, in0=s_i, scalar1=1, scalar2=None,
                    op0=ALU.logical_shift_right,
                )
                nc.vector.tensor_scalar(
                    out=r_i, in0=r_i, scalar1=MAGIC, scalar2=None, op0=ALU.subtract,
                )
                nc.vector.tensor_scalar(
                    out=r_i, in0=r_i, scalar1=-1, scalar2=None, op0=ALU.bitwise_xor,
                )
                nc.vector.tensor_scalar(
                    out=r_i, in0=r_i, scalar1=1, scalar2=None, op0=ALU.add,
                )
                t1 = stats.tile([P, NT], F32, tag="nt1")
                t1v = t1[:, 0:n]
                for _ in range(3):
                    nc.vector.tensor_mul(t1v, r_t, r_t)
                    nc.vector.tensor_mul(t1v, t1v, s_t)
                    nc.vector.tensor_scalar(
                        out=t1v, in0=t1v, scalar1=-0.5, scalar2=1.5,
                        op0=ALU.mult, op1=ALU.add,
                    )
                    nc.vector.tensor_mul(r_t, r_t, t1v)
                nc.vector.tensor_mul(nrm_all[p, blk][:, c0 : c0 + n], s_t, r_t)

            def xn_tile(p, blk, b):
                """normalize tile b, PE-transpose it into xnt."""
                lo = D * p
                xn16 = tmp.tile([P, D], BF16, tag="xn16")
                nc.vector.tensor_scalar_mul(
                    xn16[:], xb[p, blk][:, b, :], rin_all[p, blk][:, b : b + 1]
                )
                pst = ps_s.tile([P, P], BF16, tag="S")
                nc.tensor.transpose(pst[lo : lo + D, :], xn16[:], ident16[:])
                nc.vector.tensor_copy(
                    xnt[blk][lo : lo + D, b * P : (b + 1) * P], pst[lo : lo + D, :]
                )

            def xw_tile(p, blk, b):
                """xw16[:, b, 1:65] = norm_b * (xn_b @ W^T) via matmul + scale."""
                lo = D * p
                psw = ps_u.tile([P, D], F32, tag="U")
                nc.tensor.matmul(
                    psw[:],
                    lhsT=xnt[blk][lo : lo + D, b * P : (b + 1) * P],
                    rhs=wt16[blk][lo : lo + D, :],
                    start=True, stop=True,
                )
                nc.vector.tensor_scalar_mul(
                    xw16[p, blk][:, b, 1 : 1 + D], psw[:], nrm_all[p, blk][:, b : b + 1]
                )

            def prep0(p):
                """Upfront prep for block 0, chunked so the PE starts early."""
                sq = tmp.tile([P, D], F32, tag="sq")
                gpc0 = NT // NCH
                for c in range(NCH):
                    for b in range(c * gpc0, (c + 1) * gpc0):
                        nc.scalar.activation(
                            sq[:], xb[p, 0][:, b, :], AF.Square,
                            accum_out=s_all[p, 0][:, b : b + 1],
                        )
                    rsqrt_slice(p, 0, c * gpc0, gpc0)
                    for b in range(c * gpc0, (c + 1) * gpc0):
                        xn_tile(p, 0, b)
                for b in range(NT):
                    xw_tile(p, 0, b)

            def process(blk, scale, last):
                smul, sbias = _schraudolph_consts(scale)
                nsteps = NCH * NT
                U = {}        # chunk -> pair -> psum tile
                E_hist = {}   # step -> pair -> E tile
                deferred = []

                def u_mms(k):
                    a_, b_ = divmod(k, NT)
                    for p in range(npairs):
                        for h in range(2):
                            nc.tensor.matmul(
                                U[a_][p][:, h * 512 : (h + 1) * 512],
                                lhsT=xw16[p, blk][:, b_, :],
                                rhs=E_hist[k][p][:, h * 512 : (h + 1) * 512],
                                start=(b_ == 0),
                                stop=(b_ == NT - 1),
                            )
                    if k in E_hist:
                        del E_hist[k]

                def make_epilogue_pieces(a):
                    """Small deferred closures; drained a couple per step."""
                    pieces = []
                    gi0 = a * GPC
                    u16 = {}
                    T = {}
                    rz = {}
                    gm = {}

                    def mk(p):
                        def c_copy_l():
                            u16[p] = fin.tile([MW, CHW], BF16, tag="u16", name=f"u16_{blk}_{a}_{p}")
                            nc.vector.tensor_copy(u16[p][:, 0:512], U[a][p][:, 0:512])

                        def c_copy_r():
                            nc.vector.tensor_copy(u16[p][:, 512:1024], U[a][p][:, 512:1024])

                        def c_tr():
                            T[p] = fin.tile([P, GPC, MW], BF16, tag="T", name=f"T_{blk}_{a}_{p}")
                            nc.sync.dma_start_transpose(T[p][:], u16[p][:])
                            rz[p] = tmp.tile([P, GPC], F32, tag="rz", name=f"rz_{blk}_{a}_{p}")
                            nc.vector.reciprocal(rz[p][:], T[p][:, :, 0])

                        def c_mul_lo():
                            gm[p] = fin.tile([P, GPC, D], F32, tag="gm", name=f"gm_{blk}_{a}_{p}")
                            for t in range(GPC // 2):
                                nc.vector.tensor_scalar_mul(
                                    gm[p][:, t, :], T[p][:, t, 1 : 1 + D],
                                    rz[p][:, t : t + 1],
                                )

                        def c_mul_hi():
                            for t in range(GPC // 2, GPC):
                                nc.vector.tensor_scalar_mul(
                                    gm[p][:, t, :], T[p][:, t, 1 : 1 + D],
                                    rz[p][:, t : t + 1],
                                )

                        def c_add():
                            nc.vector.tensor_add(
                                gm[p][:], gm[p][:], xb[p, blk][:, gi0 : gi0 + GPC, :]
                            )

                        def c_relu():
                            if not last:
                                dst = xb[p, blk + 1][:, gi0 : gi0 + GPC, :]
                                nc.vector.tensor_scalar_max(dst, gm[p][:], 0.0)
                            else:
                                oo = fin.tile([P, GPC, D], F32, tag="oo", name=f"oo_{blk}_{a}_{p}")
                                nc.vector.tensor_scalar_max(oo[:], gm[p][:], 0.0)
                                nc.sync.dma_start(out_t[p][:, gi0 : gi0 + GPC, :], oo[:])

                        return [c_copy_l, c_copy_r, c_tr, c_mul_lo, c_mul_hi, c_add, c_relu]

                    per_p = [mk(p) for p in range(npairs)]
                    for idx in range(len(per_p[0])):
                        for p in range(npairs):
                            pieces.append(per_p[p][idx])

                    if not last:
                        # next block's norms + rsqrt + xnt tiles for this chunk
                        def mk_prep(p):
                            sub = []

                            def c_norms_lo():
                                sqo = tmp.tile([P, D], F32, tag="sqo")
                                for t in range(GPC // 2):
                                    gi = gi0 + t
                                    nc.vector._custom_dve(
                                        TENSOR_TENSOR_REDUCE,
                                        out=sqo[:],
                                        in0=xb[p, blk + 1][:, gi, :],
                                        in1=xb[p, blk + 1][:, gi, :],
                                        s0=0.0, s1=1.0,
                                        accum_out=s_all[p, blk + 1][:, gi : gi + 1],
                                    )

                            def c_norms_hi():
                                sqo = tmp.tile([P, D], F32, tag="sqo")
                                for t in range(GPC // 2, GPC):
                                    gi = gi0 + t
                                    nc.vector._custom_dve(
                                        TENSOR_TENSOR_REDUCE,
                                        out=sqo[:],
                                        in0=xb[p, blk + 1][:, gi, :],
                                        in1=xb[p, blk + 1][:, gi, :],
                                        s0=0.0, s1=1.0,
                                        accum_out=s_all[p, blk + 1][:, gi : gi + 1],
                                    )

                            def c_rsqrt():
                                rsqrt_slice(p, blk + 1, gi0, GPC)

                            sub.extend([c_norms_lo, c_norms_hi, c_rsqrt])
                            for t in range(GPC):
                                sub.append(
                                    (lambda tt: lambda: xn_tile(p, blk + 1, gi0 + tt))(t)
                                )
                            return sub

                        per_pp = [mk_prep(p) for p in range(npairs)]
                        for idx in range(len(per_pp[0])):
                            for p in range(npairs):
                                pieces.append(per_pp[p][idx])
                    return pieces

                for k in range(nsteps):
                    a, b = divmod(k, NT)
                    if b == 0:
                        U[a] = {
                            p: ps_u.tile([MW, CHW], F32, tag="U", name=f"U_{blk}_{a}_{p}")
                            for p in range(npairs)
                        }
                    # S matmuls, both pairs adjacent (different PE row groups)
                    S = {}
                    for p in range(npairs):
                        S[p] = ps_s.tile([P, CHW], F32, tag="S", name=f"S_{k}_{p}")
                    for h in range(2):
                        for p in range(npairs):
                            lo = D * p
                            nc.tensor.matmul(
                                S[p][:, h * 512 : (h + 1) * 512],
                                lhsT=xnt[blk][lo : lo + D, b * P : (b + 1) * P],
                                rhs=xnt[blk][
                                    lo : lo + D, a * CHW + h * 512 : a * CHW + (h + 1) * 512
                                ],
                                start=True, stop=True,
                            )
                    # exp: scalar engine + vector engine split
                    E_hist[k] = {}
                    for p in range(npairs):
                        Ek = epool.tile([P, CHW], FP16, tag="E", name=f"E_{k}_{p}")
                        E_hist[k][p] = Ek
                        nc.scalar.activation(
                            Ek[:, 0:ACT_W], S[p][:, 0:ACT_W], AF.Exp, scale=scale
                        )
                        nc.vector.tensor_scalar(
                            out=Ek[:, ACT_W:CHW].bitcast(U16),
                            in0=S[p][:, ACT_W:CHW],
                            scalar1=smul, scalar2=sbias,
                            op0=ALU.mult, op1=ALU.add,
                        )
                    # AV matmuls at lag
                    if k >= LAG:
                        u_mms(k - LAG)
                        kk = k - LAG
                        if kk % NT == NT - 1:
                            deferred.extend(make_epilogue_pieces(kk // NT))
                    # drain deferred pieces
                    for _ in range(2):
                        if deferred:
                            deferred.pop(0)()

                # tail
                for k in range(nsteps - LAG, nsteps):
                    u_mms(k)
                deferred.extend(make_epilogue_pieces(NCH - 1))
                while deferred:
                    deferred.pop(0)()
                if not last:
                    # xw matmuls for the next block (PSUM rings are free here)
                    for b in range(NT):
                        for p in range(npairs):
                            xw_tile(p, blk + 1, b)

            for p in range(npairs):
                prep0(p)
            for blk in range(nblocks):
                process(blk, scales[blk], last=(blk == nblocks - 1))

    nc.compile()
    return nc


_CACHE = {}


def _get_nc(scales, n_rows, npairs):
    key = (tuple(scales), n_rows, npairs)
    if key not in _CACHE:
        _CACHE[key] = build_nc(list(scales), n_rows=n_rows, npairs=npairs)
    return _CACHE[key]


def kernel(x, W1, W2, alpha1, alpha2):
    x = np.asarray(x, dtype=np.float32)
    B, H, N, d = x.shape
    assert d == D and (B * H) % N_CORES == 0
    npairs = (B * H) // N_CORES
    s1 = 1.0 / max(float(alpha1), 0.01)
    s2 = 1.0 / max(float(alpha2), 0.01)
    nc = _get_nc((s1, s2), N, npairs)

    xf = np.ascontiguousarray(x.reshape(B * H, N, d))
    w0 = np.ascontiguousarray(np.asarray(W1, dtype=np.float32).T)
    w1 = np.ascontiguousarray(np.asarray(W2, dtype=np.float32).T)
    in_maps = [
        {"xin": xf[npairs * c : npairs * (c + 1)], "w0t": w0, "w1t": w1}
        for c in range(N_CORES)
    ]
    res = run_bass_kernel_spmd(nc, in_maps, core_ids=list(range(N_CORES)))
    outs = np.stack([r["out"] for r in res.results])
    return outs.reshape(B, H, N, d).astype(np.float32)
